# revision 3
# baseline (speedup 1.0000x reference)
"""Causal self-attention (per-head A projections) on 8 TRN2 NeuronCores.

Shapes: h [B=2, T=2048, d=64] f32, A [H=8, d, d] f32.
  q = h @ A[i]; scores = q @ h^T (causal); out_i = softmax(scores) @ h.
Sharding: one head per core (embarrassingly parallel, no collectives).
Each core receives the full h and its own A[i]; host concatenates heads.

Kernel structure per core, per batch:
  Phase A: DMA h -> SBUF, TensorE-transpose into hT (65 rows: 64 data + ones
           row), qT = A^T-style matmul.  All f32.
  Stats:   scores[t,s] tiles (lhsT=qT_tile, rhs=hT chunk); causal diag handled
           by ACCUMULATING a -1e30 upper-triangular matrix via an extra
           identity-matmul into the same PSUM; DVE reduce_max -> row max m;
           -m transposed into the 65th row of the qT chunk tile.
  Pass 2:  scoresT[s,t] tiles with K=65 (fuses the -m subtraction); diag
           masked the same identity-matmul way; ACT exp reads PSUM and writes
           pT bf16 straight to SBUF (no transposes of p needed).
  AV:      out[t,:64+1] = sum_j pT_j^T @ [h_j | 1] accumulated in PSUM; the
           extra ones column yields the softmax denominator l for free.
  Final:   DVE reciprocal(l), ACT copy with per-partition scale, DMA out.
"""

import sys

for _p in ("/opt/trn_rl_repo",):
    if _p not in sys.path:
        sys.path.insert(0, _p)

import numpy as np
from contextlib import ExitStack

import concourse.bass as bass
import concourse.tile as tile
from concourse import bacc, mybir
from concourse.masks import make_identity
from concourse.bass_utils import run_bass_kernel_spmd

B, T, D, H = 2, 2048, 64, 8
P = 128                # square tile size (t and s)
NT = T // P            # 16 tiles along t/s
CH = 512               # chunk width (PSUM bank / fp32 moving max)
NCH = T // CH          # 4 chunks
NEG = -1e30

f32 = mybir.dt.float32
bf16 = mybir.dt.bfloat16

STATS_DT = f32         # dtype of the stats-pass score matmul inputs
SCORE_DT = f32         # dtype of the pass-2 score matmul inputs


def _ceil_div(a, b):
    return (a + b - 1) // b


def _build(ctx: ExitStack, tc: "tile.TileContext", h_ext, A_ext, out_ext):
    nc = tc.nc

    consts = ctx.enter_context(tc.tile_pool(name="consts", bufs=1))
    hpool = ctx.enter_context(tc.tile_pool(name="hpool", bufs=2))
    qpool = ctx.enter_context(tc.tile_pool(name="qpool", bufs=2))
    ppool = ctx.enter_context(tc.tile_pool(name="ppool", bufs=2))
    spool = ctx.enter_context(tc.tile_pool(name="spool", bufs=4))
    opool = ctx.enter_context(tc.tile_pool(name="opool", bufs=3))
    ps_stat = ctx.enter_context(tc.tile_pool(name="ps_stat", bufs=2, space="PSUM"))
    ps_p2 = ctx.enter_context(tc.tile_pool(name="ps_p2", bufs=2, space="PSUM"))
    ps_av = ctx.enter_context(tc.tile_pool(name="ps_av", bufs=2, space="PSUM"))
    ps_misc = ctx.enter_context(tc.tile_pool(name="ps_misc", bufs=2, space="PSUM"))

    # ---- constants ----
    ident = consts.tile([P, P], f32)
    make_identity(nc, ident)
    identb = consts.tile([P, P], bf16)
    make_identity(nc, identb)

    # Umask[t, s] = NEG where s > t else 0 (stats-pass diagonal tile).
    umask = consts.tile([P, P], bf16)
    nc.gpsimd.memset(umask, 0.0)
    nc.gpsimd.affine_select(
        out=umask, in_=umask, compare_op=mybir.AluOpType.is_ge, fill=NEG,
        base=0, channel_multiplier=1, pattern=[[-1, P]],
    )

    # Vk[s, t'] = NEG where s > t' - 128k else 0 (pass-2 diagonal tiles).
    vmasks = []
    for k in range(CH // P):
        vk = consts.tile([P, CH], bf16, tag=f"vmask{k}")
        nc.gpsimd.memset(vk, 0.0)
        nc.gpsimd.affine_select(
            out=vk, in_=vk, compare_op=mybir.AluOpType.is_ge, fill=NEG,
            base=-P * k, channel_multiplier=-1, pattern=[[1, CH]],
        )
        vmasks.append(vk)

    # A for this core's head: [d, e] natural layout (d on partitions).
    Asb = consts.tile([D, D], f32)
    nc.sync.dma_start(out=Asb, in_=A_ext)

    for b in range(B):
        # ---- Phase A: load h, build hT_ext, qT chunks, hs_ext ----
        hs32 = hpool.tile([P, NT, D], f32, tag="hs32")
        nc.sync.dma_start(out=hs32, in_=h_ext[b].rearrange("(j p) d -> p j d", p=P))

        hs16 = hpool.tile([P, NT, D + 1], bf16, tag="hs16")
        nc.vector.tensor_copy(hs16[:, :, 0:D], hs32)
        nc.gpsimd.memset(hs16[:, :, D : D + 1], 1.0)

        hTe = hpool.tile([D + 1, T], f32, tag="hTe")
        nc.gpsimd.memset(hTe[D : D + 1, :], 1.0)
        for j in range(NT):
            pt = ps_misc.tile([D, P], f32, tag="misc")
            nc.tensor.transpose(pt, hs32[:, j, :], ident)
            if j % 2 == 0:
                nc.scalar.copy(hTe[0:D, j * P : (j + 1) * P], pt)
            else:
                nc.vector.tensor_copy(hTe[0:D, j * P : (j + 1) * P], pt)

        # qT chunks [65, CH]: rows 0..63 = qT, row 64 = -m (written later).
        qTs = []
        for c in range(NCH):
            qc = qpool.tile([D + 1, CH], f32, tag=f"qT{c}")
            pq = ps_misc.tile([D, CH], f32, tag="misc")
            nc.tensor.matmul(
                pq, lhsT=Asb, rhs=hTe[0:D, c * CH : (c + 1) * CH],
                start=True, stop=True,
            )
            if c % 2 == 0:
                nc.scalar.copy(qc[0:D, :], pq)
            else:
                nc.vector.tensor_copy(qc[0:D, :], pq)
            qTs.append(qc)

        # ---- Stats pass: row max per t-tile ----
        for i in range(NT):
            s_end = (i + 1) * P
            nchunks = _ceil_div(s_end, CH)
            mxp = spool.tile([P, 4], f32, tag="mxp")
            lhs_q = qTs[i // 4][0:D, (i % 4) * P : (i % 4 + 1) * P]
            for c in range(nchunks):
                w = min(CH, s_end - c * CH)
                ps = ps_stat.tile([P, CH], f32, tag="stat")
                diag = c == nchunks - 1
                nc.tensor.matmul(
                    ps[:, 0:w], lhsT=lhs_q, rhs=hTe[0:D, c * CH : c * CH + w],
                    start=True, stop=not diag, skip_group_check=True,
                )
                if diag:
                    nc.tensor.matmul(
                        ps[:, w - P : w], lhsT=identb, rhs=umask,
                        start=False, stop=True, skip_group_check=True,
                    )
                nc.vector.reduce_max(
                    mxp[:, c : c + 1], ps[:, 0:w], axis=mybir.AxisListType.X
                )
            negm = spool.tile([P, 1], f32, tag="negm")
            if nchunks > 1:
                mi = spool.tile([P, 1], f32, tag="mi")
                nc.vector.reduce_max(mi, mxp[:, 0:nchunks], axis=mybir.AxisListType.X)
                nc.vector.tensor_scalar_mul(negm, mi, -1.0)
            else:
                nc.vector.tensor_scalar_mul(negm, mxp[:, 0:1], -1.0)
            pm = ps_misc.tile([1, P], f32, tag="misc")
            nc.tensor.transpose(pm, negm, ident)
            nc.vector.tensor_copy(
                qTs[i // 4][D : D + 1, (i % 4) * P : (i % 4 + 1) * P], pm
            )

        # ---- Pass 2 + AV per chunk ----
        for c in range(NCH):
            jmax = 4 * c + 3
            pTs = []
            for j in range(jmax + 1):
                p2 = ps_p2.tile([P, CH], f32, tag="p2")
                nc.tensor.matmul(
                    p2, lhsT=hTe[:, j * P : (j + 1) * P], rhs=qTs[c],
                    start=True, stop=(j < 4 * c), skip_group_check=True,
                )
                if j >= 4 * c:
                    nc.tensor.matmul(
                        p2, lhsT=identb, rhs=vmasks[j - 4 * c],
                        start=False, stop=True, skip_group_check=True,
                    )
                pT = ppool.tile([P, CH], bf16, tag=f"pT{j}")
                nc.scalar.activation(pT, p2, mybir.ActivationFunctionType.Exp)
                pTs.append(pT)
            for ii in range(4 * c, 4 * c + 4):
                pav = ps_av.tile([P, D + 1], f32, tag="av")
                tsl = (ii - 4 * c) * P
                for j in range(ii + 1):
                    nc.tensor.matmul(
                        pav, lhsT=pTs[j][:, tsl : tsl + P], rhs=hs16[:, j, :],
                        start=(j == 0), stop=(j == ii), skip_group_check=True,
                    )
                rl = spool.tile([P, 1], f32, tag="rl")
                nc.vector.reciprocal(rl, pav[:, D : D + 1])
                osb = opool.tile([P, D], f32, tag="osb")
                nc.scalar.mul(osb, pav[:, 0:D], rl)
                nc.sync.dma_start(
                    out=out_ext[b, ii * P : (ii + 1) * P, :], in_=osb
                )


_cache = {}


def _get_nc():
    if "nc" not in _cache:
        nc = bacc.Bacc(
            "TRN2", target_bir_lowering=False, debug=False, num_devices=H
        )
        h_ext = nc.dram_tensor("h", [B, T, D], f32, kind="ExternalInput").ap()
        A_ext = nc.dram_tensor("A", [D, D], f32, kind="ExternalInput").ap()
        out_ext = nc.dram_tensor("out", [B, T, D], f32, kind="ExternalOutput").ap()
        with tile.TileContext(nc) as tc:
            with ExitStack() as ctx:
                _build(ctx, tc, h_ext, A_ext, out_ext)
        nc.compile()
        _cache["nc"] = nc
    return _cache["nc"]


def run(h, A, **kw):
    """Run on hardware; returns (full output [B,T,H*D], BassKernelResults)."""
    nc = _get_nc()
    h = np.ascontiguousarray(h, dtype=np.float32)
    A = np.ascontiguousarray(A, dtype=np.float32)
    in_maps = [{"h": h, "A": np.ascontiguousarray(A[i])} for i in range(H)]
    res = run_bass_kernel_spmd(nc, in_maps, core_ids=list(range(H)), **kw)
    out = np.concatenate([res.results[i]["out"] for i in range(H)], axis=-1)
    return out, res


def kernel(h, A):
    out, _ = run(h, A)
    return out


# revision 12
# speedup vs baseline: 2.5281x; 2.5281x over previous
"""Causal self-attention (per-head A projections) on 8 TRN2 NeuronCores.

Shapes: h [B=2, T=2048, d=64] f32, A [H=8, d, d] f32.
  q = h @ A[i]; scores = q @ h^T (causal); out_i = softmax(scores) @ h.
Sharding: one head per core (embarrassingly parallel, no collectives).
Each core receives the full h and its own A[i]; host concatenates heads.

Kernel structure per core, per batch:
  Phase A: DMA h -> SBUF, TensorE-transpose into hT (65 rows: 64 data + ones
           row), qT = A^T-style matmul.  All f32.
  Stats:   scores[t,s] tiles (lhsT=qT_tile, rhs=hT chunk); causal diag handled
           by ACCUMULATING a -1e30 upper-triangular matrix via an extra
           identity-matmul into the same PSUM; DVE reduce_max -> row max m;
           -m transposed into the 65th row of the qT chunk tile.
  Pass 2:  scoresT[s,t] tiles with K=65 (fuses the -m subtraction); diag
           masked the same identity-matmul way; ACT exp reads PSUM and writes
           pT bf16 straight to SBUF (no transposes of p needed).
  AV:      out[t,:64+1] = sum_j pT_j^T @ [h_j | 1] accumulated in PSUM; the
           extra ones column yields the softmax denominator l for free.
  Final:   DVE reciprocal(l), ACT copy with per-partition scale, DMA out.
"""

import sys

for _p in ("/opt/trn_rl_repo",):
    if _p not in sys.path:
        sys.path.insert(0, _p)

import numpy as np
from contextlib import ExitStack

import concourse.bass as bass
import concourse.tile as tile
from concourse import bacc, mybir
from concourse.masks import make_identity
from concourse.bass_utils import run_bass_kernel_spmd

B, T, D, H = 2, 2048, 64, 8
P = 128                # square tile size (t and s)
NT = T // P            # 16 tiles along t/s
CH = 512               # chunk width (PSUM bank / fp32 moving max)
NCH = T // CH          # 4 chunks
NEG = -1e30

f32 = mybir.dt.float32
f32r = mybir.dt.float32r
bf16 = mybir.dt.bfloat16

# Pass-2 score matmul mode: "f32" (4 cyc/row, exact), "f32r" (1 cyc/row,
# reduced precision single-pass).  Stats pass is always bf16 (only feeds the
# row-max bound, where +-1 error is harmless).
PASS2_MODE = "f32r"


def _ceil_div(a, b):
    return (a + b - 1) // b


def _build(ctx: ExitStack, tc: "tile.TileContext", h_ext, A_ext, out_ext):
    nc = tc.nc

    consts = ctx.enter_context(tc.tile_pool(name="consts", bufs=1))
    hpool = ctx.enter_context(tc.tile_pool(name="hpool", bufs=2))
    qpool = ctx.enter_context(tc.tile_pool(name="qpool", bufs=2))
    ppool = ctx.enter_context(tc.tile_pool(name="ppool", bufs=2))
    spool = ctx.enter_context(tc.tile_pool(name="spool", bufs=4))
    opool = ctx.enter_context(tc.tile_pool(name="opool", bufs=3))
    ps_stat = ctx.enter_context(tc.tile_pool(name="ps_stat", bufs=2, space="PSUM"))
    ps_p2 = ctx.enter_context(tc.tile_pool(name="ps_p2", bufs=2, space="PSUM"))
    ps_av = ctx.enter_context(tc.tile_pool(name="ps_av", bufs=2, space="PSUM"))
    ps_misc = ctx.enter_context(tc.tile_pool(name="ps_misc", bufs=2, space="PSUM"))

    # ---- constants ----
    ident = consts.tile([P, P], f32)
    make_identity(nc, ident)
    identb = consts.tile([P, P], bf16)
    make_identity(nc, identb)

    # Umask[t, s] = NEG where s > t else 0 (stats-pass diagonal tile).
    umask = consts.tile([P, P], bf16)
    nc.gpsimd.memset(umask, 0.0)
    nc.gpsimd.affine_select(
        out=umask, in_=umask, compare_op=mybir.AluOpType.is_ge, fill=NEG,
        base=0, channel_multiplier=1, pattern=[[-1, P]],
    )

    # Vk[s, t'] = NEG where s > t' - 128k else 0 (pass-2 diagonal tiles).
    vmasks = []
    for k in range(CH // P):
        vk = consts.tile([P, CH], bf16, tag=f"vmask{k}")
        nc.gpsimd.memset(vk, 0.0)
        nc.gpsimd.affine_select(
            out=vk, in_=vk, compare_op=mybir.AluOpType.is_ge, fill=NEG,
            base=-P * k, channel_multiplier=-1, pattern=[[1, CH]],
        )
        vmasks.append(vk)

    # A for this core's head: [d, e] natural layout (d on partitions).
    p2dt = f32r if PASS2_MODE == "f32r" else f32
    Asb32 = consts.tile([D, D], f32)
    nc.sync.dma_start(out=Asb32, in_=A_ext)
    if PASS2_MODE == "f32r":
        Asb = consts.tile([D, D], f32r)
        nc.scalar.copy(Asb, Asb32)
    else:
        Asb = Asb32

    for b in range(B):
        # ---- Phase A: load h, build hT_ext, qT chunks, hs_ext ----
        hs32 = hpool.tile([P, NT, D + 1], f32, tag="hs32")
        nc.sync.dma_start(
            out=hs32[:, :, 0:D], in_=h_ext[b].rearrange("(j p) d -> p j d", p=P)
        )
        nc.gpsimd.memset(hs32[:, :, D : D + 1], 1.0)

        hs16 = hpool.tile([P, NT, D + 1], bf16, tag="hs16")
        nc.vector.tensor_copy(hs16, hs32)

        hTe = hpool.tile([D + 1, T], p2dt, tag="hTe")
        for j in range(NT):
            pt = ps_misc.tile([D + 1, P], f32, tag="misc")
            nc.tensor.transpose(pt, hs32[:, j, :], ident)
            if j % 2 == 0:
                nc.scalar.copy(hTe[:, j * P : (j + 1) * P], pt)
            else:
                nc.vector.tensor_copy(hTe[:, j * P : (j + 1) * P], pt)

        # bf16 copy of hT for the stats pass.
        hT16 = hpool.tile([D, T], bf16, tag="hT16")
        nc.vector.tensor_copy(hT16, hTe[0:D, :])

        # qT chunks [65, CH]: rows 0..63 = qT, row 64 = -m (written later).
        qTs = []
        qT16s = []
        for c in range(NCH):
            qc = qpool.tile([D + 1, CH], p2dt, tag=f"qT{c}")
            pq = ps_misc.tile([D, CH], f32, tag="misc")
            nc.tensor.matmul(
                pq, lhsT=Asb, rhs=hTe[0:D, c * CH : (c + 1) * CH],
                start=True, stop=True,
            )
            if c % 2 == 0:
                nc.scalar.copy(qc[0:D, :], pq)
            else:
                nc.vector.tensor_copy(qc[0:D, :], pq)
            qc16 = qpool.tile([D, CH], bf16, tag=f"qT16_{c}")
            nc.scalar.copy(qc16, pq)
            qTs.append(qc)
            qT16s.append(qc16)

        # ---- Stats pass: row max per t-tile ----
        for i in range(NT):
            s_end = (i + 1) * P
            nchunks = _ceil_div(s_end, CH)
            mxp = spool.tile([P, 4], f32, tag="mxp")
            lhs_q = qT16s[i // 4][:, (i % 4) * P : (i % 4 + 1) * P]
            for c in range(nchunks):
                w = min(CH, s_end - c * CH)
                ps = ps_stat.tile([P, CH], f32, tag="stat")
                diag = c == nchunks - 1
                nc.tensor.matmul(
                    ps[:, 0:w], lhsT=lhs_q, rhs=hT16[:, c * CH : c * CH + w],
                    start=True, stop=not diag, skip_group_check=True,
                )
                if diag:
                    nc.tensor.matmul(
                        ps[:, w - P : w], lhsT=identb, rhs=umask,
                        start=False, stop=True, skip_group_check=True,
                    )
                nc.vector.reduce_max(
                    mxp[:, c : c + 1], ps[:, 0:w], axis=mybir.AxisListType.X
                )
            negm = spool.tile([P, 1], f32, tag="negm")
            if nchunks > 1:
                mi = spool.tile([P, 1], f32, tag="mi")
                nc.vector.reduce_max(mi, mxp[:, 0:nchunks], axis=mybir.AxisListType.X)
                nc.vector.tensor_scalar_mul(negm, mi, -1.0)
            else:
                nc.vector.tensor_scalar_mul(negm, mxp[:, 0:1], -1.0)
            pm = ps_misc.tile([1, P], f32, tag="misc")
            nc.tensor.transpose(pm, negm, ident)
            nc.vector.tensor_copy(
                qTs[i // 4][D : D + 1, (i % 4) * P : (i % 4 + 1) * P], pm
            )

        # ---- Pass 2 + AV per chunk ----
        for c in range(NCH):
            jmax = 4 * c + 3
            pTs = []
            for j in range(jmax + 1):
                p2 = ps_p2.tile([P, CH], f32, tag="p2")
                nc.tensor.matmul(
                    p2, lhsT=hTe[:, j * P : (j + 1) * P], rhs=qTs[c][:, :],
                    start=True, stop=(j < 4 * c), skip_group_check=True,
                )
                if j >= 4 * c:
                    nc.tensor.matmul(
                        p2, lhsT=identb, rhs=vmasks[j - 4 * c],
                        start=False, stop=True, skip_group_check=True,
                    )
                pT = ppool.tile([P, CH], bf16, tag=f"pT{j}")
                nc.scalar.activation(pT, p2, mybir.ActivationFunctionType.Exp)
                pTs.append(pT)
            for ii in range(4 * c, 4 * c + 4):
                pav = ps_av.tile([P, D + 1], f32, tag="av")
                tsl = (ii - 4 * c) * P
                for j in range(ii + 1):
                    nc.tensor.matmul(
                        pav, lhsT=pTs[j][:, tsl : tsl + P], rhs=hs16[:, j, :],
                        start=(j == 0), stop=(j == ii), skip_group_check=True,
                    )
                rl = spool.tile([P, 1], f32, tag="rl")
                nc.vector.reciprocal(rl, pav[:, D : D + 1])
                osb = opool.tile([P, D], f32, tag="osb")
                nc.scalar.mul(osb, pav[:, 0:D], rl)
                nc.sync.dma_start(
                    out=out_ext[b, ii * P : (ii + 1) * P, :], in_=osb
                )


_cache = {}


def _get_nc():
    if "nc" not in _cache:
        nc = bacc.Bacc(
            "TRN2", target_bir_lowering=False, debug=False, num_devices=H
        )
        h_ext = nc.dram_tensor("h", [B, T, D], f32, kind="ExternalInput").ap()
        A_ext = nc.dram_tensor("A", [D, D], f32, kind="ExternalInput").ap()
        out_ext = nc.dram_tensor("out", [B, T, D], f32, kind="ExternalOutput").ap()
        with tile.TileContext(nc) as tc:
            with ExitStack() as ctx:
                _build(ctx, tc, h_ext, A_ext, out_ext)
        nc.compile()
        _cache["nc"] = nc
    return _cache["nc"]


def run(h, A, **kw):
    """Run on hardware; returns (full output [B,T,H*D], BassKernelResults)."""
    nc = _get_nc()
    h = np.ascontiguousarray(h, dtype=np.float32)
    A = np.ascontiguousarray(A, dtype=np.float32)
    in_maps = [{"h": h, "A": np.ascontiguousarray(A[i])} for i in range(H)]
    res = run_bass_kernel_spmd(nc, in_maps, core_ids=list(range(H)), **kw)
    out = np.concatenate([res.results[i]["out"] for i in range(H)], axis=-1)
    return out, res


def kernel(h, A):
    out, _ = run(h, A)
    return out


# revision 14
# speedup vs baseline: 2.6803x; 1.0602x over previous
"""Causal self-attention (per-head A projections) on 8 TRN2 NeuronCores.

Shapes: h [B=2, T=2048, d=64] f32, A [H=8, d, d] f32.
  q = h @ A[i]; scores = q @ h^T (causal); out_i = softmax(scores) @ h.
Sharding: one head per core (embarrassingly parallel, no collectives).
Each core receives the full h and its own A[i]; host concatenates heads.

Kernel structure per core, per batch:
  Phase A: DMA h -> SBUF, TensorE-transpose into hT (65 rows: 64 data + ones
           row), qT = A^T-style matmul.  All f32.
  Stats:   scores[t,s] tiles (lhsT=qT_tile, rhs=hT chunk); causal diag handled
           by ACCUMULATING a -1e30 upper-triangular matrix via an extra
           identity-matmul into the same PSUM; DVE reduce_max -> row max m;
           -m transposed into the 65th row of the qT chunk tile.
  Pass 2:  scoresT[s,t] tiles with K=65 (fuses the -m subtraction); diag
           masked the same identity-matmul way; ACT exp reads PSUM and writes
           pT bf16 straight to SBUF (no transposes of p needed).
  AV:      out[t,:64+1] = sum_j pT_j^T @ [h_j | 1] accumulated in PSUM; the
           extra ones column yields the softmax denominator l for free.
  Final:   DVE reciprocal(l), ACT copy with per-partition scale, DMA out.
"""

import sys

for _p in ("/opt/trn_rl_repo",):
    if _p not in sys.path:
        sys.path.insert(0, _p)

import numpy as np
from contextlib import ExitStack

import concourse.bass as bass
import concourse.tile as tile
from concourse import bacc, mybir
from concourse.masks import make_identity
from concourse.bass_utils import run_bass_kernel_spmd

B, T, D, H = 2, 2048, 64, 8
P = 128                # square tile size (t and s)
NT = T // P            # 16 tiles along t/s
CH = 512               # chunk width (PSUM bank / fp32 moving max)
NCH = T // CH          # 4 chunks
NEG = -1e30

f32 = mybir.dt.float32
f32r = mybir.dt.float32r
bf16 = mybir.dt.bfloat16

# Pass-2 score matmul mode: "f32" (4 cyc/row, exact), "f32r" (1 cyc/row,
# reduced precision single-pass).  Stats pass is always bf16 (only feeds the
# row-max bound, where +-1 error is harmless).
PASS2_MODE = "f32r"


def _ceil_div(a, b):
    return (a + b - 1) // b


def _build(ctx: ExitStack, tc: "tile.TileContext", h_ext, A_ext, out_ext):
    nc = tc.nc

    consts = ctx.enter_context(tc.tile_pool(name="consts", bufs=1))
    hpool = ctx.enter_context(tc.tile_pool(name="hpool", bufs=2))
    qpool = ctx.enter_context(tc.tile_pool(name="qpool", bufs=2))
    ppool = ctx.enter_context(tc.tile_pool(name="ppool", bufs=2))
    spool = ctx.enter_context(tc.tile_pool(name="spool", bufs=4))
    opool = ctx.enter_context(tc.tile_pool(name="opool", bufs=3))
    ps_stat = ctx.enter_context(tc.tile_pool(name="ps_stat", bufs=2, space="PSUM"))
    ps_p2 = ctx.enter_context(tc.tile_pool(name="ps_p2", bufs=2, space="PSUM"))
    ps_av = ctx.enter_context(tc.tile_pool(name="ps_av", bufs=2, space="PSUM"))
    ps_misc = ctx.enter_context(tc.tile_pool(name="ps_misc", bufs=2, space="PSUM"))

    # ---- constants ----
    ident = consts.tile([P, P], f32)
    make_identity(nc, ident)
    identb = consts.tile([P, P], bf16)
    make_identity(nc, identb)

    # Umask[t, s] = NEG where s > t else 0 (stats-pass diagonal tile).
    umask = consts.tile([P, P], bf16)
    nc.gpsimd.memset(umask, 0.0)
    nc.gpsimd.affine_select(
        out=umask, in_=umask, compare_op=mybir.AluOpType.is_ge, fill=NEG,
        base=0, channel_multiplier=1, pattern=[[-1, P]],
    )

    # Vk[s, t'] = NEG where s > t' - 128k else 0 (pass-2 diagonal tiles).
    vmasks = []
    for k in range(CH // P):
        vk = consts.tile([P, CH], bf16, tag=f"vmask{k}")
        nc.gpsimd.memset(vk, 0.0)
        nc.gpsimd.affine_select(
            out=vk, in_=vk, compare_op=mybir.AluOpType.is_ge, fill=NEG,
            base=-P * k, channel_multiplier=-1, pattern=[[1, CH]],
        )
        vmasks.append(vk)

    # A for this core's head: [d, e] natural layout (d on partitions).
    p2dt = f32r if PASS2_MODE == "f32r" else f32
    Asb32 = consts.tile([D, D], f32)
    nc.sync.dma_start(out=Asb32, in_=A_ext)
    if PASS2_MODE == "f32r":
        Asb = consts.tile([D, D], f32r)
        nc.scalar.copy(Asb, Asb32)
    else:
        Asb = Asb32

    for b in range(B):
        # ---- Phase A: load h, build hT_ext, qT chunks, hs_ext ----
        hs32 = hpool.tile([P, NT, D + 1], f32, tag="hs32")
        nc.sync.dma_start(
            out=hs32[:, :, 0:D], in_=h_ext[b].rearrange("(j p) d -> p j d", p=P)
        )
        nc.gpsimd.memset(hs32[:, :, D : D + 1], 1.0)

        hs16 = hpool.tile([P, NT, D + 1], bf16, tag="hs16")
        nc.vector.tensor_copy(hs16, hs32)

        hTe = hpool.tile([D + 1, T], p2dt, tag="hTe")
        for j in range(NT):
            pt = ps_misc.tile([D + 1, P], f32, tag="misc")
            nc.tensor.transpose(pt, hs32[:, j, :], ident)
            if j % 2 == 0:
                nc.scalar.copy(hTe[:, j * P : (j + 1) * P], pt)
            else:
                nc.vector.tensor_copy(hTe[:, j * P : (j + 1) * P], pt)

        # bf16 copy of hT for the stats pass.
        hT16 = hpool.tile([D, T], bf16, tag="hT16")
        nc.vector.tensor_copy(hT16, hTe[0:D, :])

        # qT chunks [65, CH]: rows 0..63 = qT, row 64 = -m (written later).
        qTs = []
        qT16s = []
        for c in range(NCH):
            qc = qpool.tile([D + 1, CH], p2dt, tag=f"qT{c}")
            pq = ps_misc.tile([D, CH], f32, tag="misc")
            nc.tensor.matmul(
                pq, lhsT=Asb, rhs=hTe[0:D, c * CH : (c + 1) * CH],
                start=True, stop=True,
            )
            if c % 2 == 0:
                nc.scalar.copy(qc[0:D, :], pq)
            else:
                nc.vector.tensor_copy(qc[0:D, :], pq)
            qc16 = qpool.tile([D, CH], bf16, tag=f"qT16_{c}")
            nc.scalar.copy(qc16, pq)
            qTs.append(qc)
            qT16s.append(qc16)

        # ---- Stats pass: row max per t-tile ----
        for i in range(NT):
            s_end = (i + 1) * P
            nchunks = _ceil_div(s_end, CH)
            mxp = spool.tile([P, 4], f32, tag="mxp")
            lhs_q = qT16s[i // 4][:, (i % 4) * P : (i % 4 + 1) * P]
            for c in range(nchunks):
                w = min(CH, s_end - c * CH)
                ps = ps_stat.tile([P, CH], f32, tag="stat")
                diag = c == nchunks - 1
                nc.tensor.matmul(
                    ps[:, 0:w], lhsT=lhs_q, rhs=hT16[:, c * CH : c * CH + w],
                    start=True, stop=not diag, skip_group_check=True,
                )
                if diag:
                    nc.tensor.matmul(
                        ps[:, w - P : w], lhsT=identb, rhs=umask,
                        start=False, stop=True, skip_group_check=True,
                    )
                nc.vector.reduce_max(
                    mxp[:, c : c + 1], ps[:, 0:w], axis=mybir.AxisListType.X
                )
            negm = spool.tile([P, 1], p2dt, tag="negm")
            if nchunks > 1:
                mi = spool.tile([P, 1], f32, tag="mi")
                nc.vector.reduce_max(mi, mxp[:, 0:nchunks], axis=mybir.AxisListType.X)
                nc.vector.tensor_scalar_mul(negm, mi, -1.0)
            else:
                nc.vector.tensor_scalar_mul(negm, mxp[:, 0:1], -1.0)
            # Partition-column -> free-row reshape via a tiny SBUF->SBUF DMA
            # (keeping this off TensorE preserves HAM warm-up).
            nc.sync.dma_start(
                out=qTs[i // 4][D : D + 1, (i % 4) * P : (i % 4 + 1) * P],
                in_=negm,
            )

        # ---- Pass 2 + AV per chunk ----
        for c in range(NCH):
            jmax = 4 * c + 3
            pTs = []
            for j in range(jmax + 1):
                p2 = ps_p2.tile([P, CH], f32, tag="p2")
                nc.tensor.matmul(
                    p2, lhsT=hTe[:, j * P : (j + 1) * P], rhs=qTs[c][:, :],
                    start=True, stop=(j < 4 * c), skip_group_check=True,
                )
                if j >= 4 * c:
                    nc.tensor.matmul(
                        p2, lhsT=identb, rhs=vmasks[j - 4 * c],
                        start=False, stop=True, skip_group_check=True,
                    )
                pT = ppool.tile([P, CH], bf16, tag=f"pT{j}")
                nc.scalar.activation(pT, p2, mybir.ActivationFunctionType.Exp)
                pTs.append(pT)
            for ii in range(4 * c, 4 * c + 4):
                pav = ps_av.tile([P, D + 1], f32, tag="av")
                tsl = (ii - 4 * c) * P
                for j in range(ii + 1):
                    nc.tensor.matmul(
                        pav, lhsT=pTs[j][:, tsl : tsl + P], rhs=hs16[:, j, :],
                        start=(j == 0), stop=(j == ii), skip_group_check=True,
                    )
                rl = spool.tile([P, 1], f32, tag="rl")
                nc.vector.reciprocal(rl, pav[:, D : D + 1])
                osb = opool.tile([P, D], f32, tag="osb")
                nc.scalar.mul(osb, pav[:, 0:D], rl)
                nc.sync.dma_start(
                    out=out_ext[b, ii * P : (ii + 1) * P, :], in_=osb
                )


_cache = {}


def _get_nc():
    if "nc" not in _cache:
        nc = bacc.Bacc(
            "TRN2", target_bir_lowering=False, debug=False, num_devices=H
        )
        h_ext = nc.dram_tensor("h", [B, T, D], f32, kind="ExternalInput").ap()
        A_ext = nc.dram_tensor("A", [D, D], f32, kind="ExternalInput").ap()
        out_ext = nc.dram_tensor("out", [B, T, D], f32, kind="ExternalOutput").ap()
        with tile.TileContext(nc) as tc:
            with ExitStack() as ctx:
                _build(ctx, tc, h_ext, A_ext, out_ext)
        nc.compile()
        _cache["nc"] = nc
    return _cache["nc"]


def run(h, A, **kw):
    """Run on hardware; returns (full output [B,T,H*D], BassKernelResults)."""
    nc = _get_nc()
    h = np.ascontiguousarray(h, dtype=np.float32)
    A = np.ascontiguousarray(A, dtype=np.float32)
    in_maps = [{"h": h, "A": np.ascontiguousarray(A[i])} for i in range(H)]
    res = run_bass_kernel_spmd(nc, in_maps, core_ids=list(range(H)), **kw)
    out = np.concatenate([res.results[i]["out"] for i in range(H)], axis=-1)
    return out, res


def kernel(h, A):
    out, _ = run(h, A)
    return out
